# revision 48
# baseline (speedup 1.0000x reference)
"""MessagePassingConvolution kernel for 8 Trainium2 NeuronCores.

Strategy (no collectives; ~347us profiled on HW):
  - Host: sort edges by receiver; shard by receiver windows. Core m owns
    nodes [m*1280, (m+1)*1280) = 10 windows of 128 nodes. Each window's
    edge list is padded to a fixed budget (2176 = 17 subtiles of 128) so
    the SPMD program is identical across cores. The sender gather
    (node_feats[senders]) is done host-side.
  - Einsum u[e,lo] = sum_{k,i} h[e,k] x[e,i] Wgen[k,lo,i] runs as 16
    chunk matmuls over a materialized outer product A[e,(k,i)].  The
    (k,i)->partition/chunk mapping is chosen to minimize operand
    replication DMA: k = 4*ks + ck, i = 8*ci + is, partition p = 8ks+is,
    chunk (ck,ci).  h3 is replicated 8x (DRAM-bounce broadcast, 1KB/edge)
    and x 16x (free: baked into the host-gathered stream); the DVE builds
    A in one bf16 2x-mode op per 512 edges with both factors read via
    free-dim broadcasts.  The broadcast DMA's source APs put a >=16-size
    dim outermost, which is what makes the DGE spread descriptors over
    all 16 SDMA engines (outer dim 4 -> only 4 engines, 4x slower).
  - All matmuls bf16 (fp32 matmul runs LOW/HIGH double passes, ~3x cost).
  - Per (window, 1024-edge pair), software-pipelined with a 1-pair skew
    (stage A: loads + MLP + bounce; stage B: outer product + einsum +
    transposes) so load chains complete while the previous pair computes:
      MLP (3 bf16 matmuls + Silu) -> h3p [64, e] bf16
      h3mid[p, ck, e] = h3p[4ks+ck, e] via one broadcast DMA per pair
      A = h3mid*x (DVE), u[96,*] += Wg_c.T @ A_c (16 matmuls/512)
      tensor-engine transpose -> ut (bf16), msgs = ut * attrs (DVE)
      scatter: psum[128n, 288] += S.T @ msgs, S built on-device from
        recv_local via iota==scalar compare (one DVE op per window)
    The window's tail-subtile msgs + scatter + store are deferred into
    the next window so the DVE never stalls on the tail chain.
  - Output: per-core [1280, 288] slices -> concat -> [10000, 32, 9].
"""

import sys
import numpy as np
from contextlib import ExitStack

sys.path.insert(0, "/opt/trn_rl_repo")

import concourse.bass as bass  # noqa: E402
import concourse.bacc as bacc  # noqa: E402
import concourse.mybir as mybir  # noqa: E402
import concourse.tile as tile  # noqa: E402
from concourse.masks import make_identity  # noqa: E402
from concourse.bass_utils import run_bass_kernel_spmd  # noqa: E402

import ml_dtypes  # noqa: E402

BF16 = ml_dtypes.bfloat16

# ---- problem constants (hardcoded per spec) ----
N_NODES = 10000
N_EDGES = 160000
C = 32
RADIAL = 8
HID = 64
NL = 3
L_DIMS = (1, 3, 5)
NSH = 9  # sum(L_DIMS)
AVG_NUM_NEIGHBORS = 16.0

N_CORES = 8
WIN = 128                      # nodes per window (psum partitions)
WINS_PER_CORE = 10
NODES_PER_CORE = WIN * WINS_PER_CORE     # 1280
N_NODES_PAD = NODES_PER_CORE * N_CORES   # 10240
SUB = 128                      # edges per subtile
SUBS_PER_WIN = 17              # window edge budget = 2176 (data max 2155)
WIN_E = SUB * SUBS_PER_WIN     # 2176
E_CORE = WIN_E * WINS_PER_CORE  # 21760
N_ST = E_CORE // SUB           # 170 subtiles per core
TILE_SIZES = (512, 512, 512, 512, 128)   # per-window einsum tiles
N_CHUNK = 16                   # ki chunks (2048 / 128)
LO = NL * C                    # 96
F_OUT = NSH * C                # 288

FP32 = mybir.dt.float32
BF16_DT = mybir.dt.bfloat16

_CACHED = {}

# CoreSim doesn't implement Silu; sim_test.py overrides this to Sigmoid and
# checks against a sigmoid-variant reference to validate the data plumbing.
ACT_FUNC = mybir.ActivationFunctionType.Silu


def _build_nc():
    nc = bacc.Bacc()

    ef = nc.dram_tensor("ef", [RADIAL, E_CORE], BF16_DT, kind="ExternalInput")
    x4 = nc.dram_tensor("x4", [128, 4 * E_CORE], BF16_DT, kind="ExternalInput")
    at = nc.dram_tensor("at", [SUB, N_ST * NSH], BF16_DT,
                        kind="ExternalInput")
    rl = nc.dram_tensor("rl", [SUB, N_ST], FP32, kind="ExternalInput")
    w1 = nc.dram_tensor("w1", [RADIAL, HID], BF16_DT, kind="ExternalInput")
    w2 = nc.dram_tensor("w2", [HID, HID], BF16_DT, kind="ExternalInput")
    w3 = nc.dram_tensor("w3", [HID, HID], BF16_DT, kind="ExternalInput")
    wg = nc.dram_tensor("wg", [128, N_CHUNK * LO], BF16_DT, kind="ExternalInput")
    iota = nc.dram_tensor("iota", [128, 128], FP32, kind="ExternalInput")
    out = nc.dram_tensor("out", [NODES_PER_CORE, F_OUT], FP32, kind="ExternalOutput")

    with tile.TileContext(nc) as tc, ExitStack() as ctx:
        const_p = ctx.enter_context(tc.tile_pool(name="const", bufs=1))
        stream_p = ctx.enter_context(tc.tile_pool(name="stream", bufs=3))
        win_p = ctx.enter_context(tc.tile_pool(name="win", bufs=2))
        chunk_p = ctx.enter_context(tc.tile_pool(name="chunk", bufs=3))
        bc_p = ctx.enter_context(tc.tile_pool(name="bc", bufs=2))
        psum_mlp = ctx.enter_context(tc.tile_pool(name="pmlp", bufs=2, space="PSUM"))
        psum_u = ctx.enter_context(tc.tile_pool(name="pu", bufs=3, space="PSUM"))
        psum_ut = ctx.enter_context(tc.tile_pool(name="put", bufs=2, space="PSUM"))
        psum_acc = ctx.enter_context(tc.tile_pool(name="pacc", bufs=1, space="PSUM"))
        dram_p = ctx.enter_context(tc.tile_pool(name="dram", bufs=3, space="DRAM"))
        tail_p = ctx.enter_context(tc.tile_pool(name="tail", bufs=1))

        # ---- one-time constants into SBUF ----
        w1_sb = const_p.tile([RADIAL, HID], BF16_DT)
        nc.scalar.dma_start(w1_sb[:], w1[:])
        w2_sb = const_p.tile([HID, HID], BF16_DT)
        nc.scalar.dma_start(w2_sb[:], w2[:])
        w3_sb = const_p.tile([HID, HID], BF16_DT)
        nc.scalar.dma_start(w3_sb[:], w3[:])
        wg_sb = const_p.tile([128, N_CHUNK * LO], BF16_DT)
        nc.scalar.dma_start(wg_sb[:], wg[:])
        iota_sb = const_p.tile([128, 128], FP32)
        nc.scalar.dma_start(iota_sb[:], iota[:])
        ident_sb = const_p.tile([128, 128], BF16_DT)
        make_identity(nc, ident_sb[:])

        lofs = (0, 1, 4)

        def msgs_range(f_ut, f_at, f_msgs, s0, s1):
            # msgs = ut * attr for subtiles [s0, s1) (attr bcast along c)
            ns = s1 - s0
            for l in range(NL):
                dim = L_DIMS[l]
                u_ap = f_ut[:, s0:s1, None, l * C:(l + 1) * C].to_broadcast(
                    [SUB, ns, dim, C])
                a_ap = f_at[:].rearrange("p (s m) -> p s m", m=NSH)[
                    :, s0:s1, lofs[l]:lofs[l] + dim]
                a_ap = a_ap[:, :, :, None].to_broadcast([SUB, ns, dim, C])
                nc.vector.tensor_tensor(
                    out=f_msgs[:, s0:s1, lofs[l] * C:(lofs[l] + dim) * C]
                    .rearrange("p s (m c) -> p s m c", c=C),
                    in0=u_ap, in1=a_ap, op=mybir.AluOpType.mult)

        def flush_window(fw, f_ut, f_at, f_msgs, f_s):
            # tail subtile's msgs, then scatter + store
            msgs_range(f_ut, f_at, f_msgs, SUBS_PER_WIN - 1, SUBS_PER_WIN)
            # scatter: psum_out += S.T @ msgs per subtile
            acc = psum_acc.tile([WIN, F_OUT], FP32, tag="acc")
            for st in range(SUBS_PER_WIN):
                nc.tensor.matmul(out=acc[:], lhsT=f_s[:, st, :],
                                 rhs=f_msgs[:, st, :],
                                 start=(st == 0), stop=(st == SUBS_PER_WIN - 1),
                                 skip_group_check=True)
            out_sb = stream_p.tile([WIN, F_OUT], FP32, tag="osb")
            nc.scalar.copy(out_sb[:], acc[:])
            nc.scalar.dma_start(out[fw * WIN:(fw + 1) * WIN, :], out_sb[:])

        # ---- flat skewed pipeline over all (window, pair) tiles ----
        # stage A issues DMAs + MLP + bounce for a pair; stage B (one step
        # later) does the outer product + einsum + transposes, so each
        # pair's load chain completes while the previous pair computes.
        win_tiles = {}

        def win_start(w):
            at_sb = win_p.tile([SUB, SUBS_PER_WIN * NSH], BF16_DT, tag="at")
            nc.scalar.dma_start(
                at_sb[:],
                at[:, w * SUBS_PER_WIN * NSH:(w + 1) * SUBS_PER_WIN * NSH])
            rl_sb = win_p.tile([SUB, SUBS_PER_WIN], FP32, tag="rl")
            nc.scalar.dma_start(
                rl_sb[:], rl[:, w * SUBS_PER_WIN:(w + 1) * SUBS_PER_WIN])
            ut_sb = win_p.tile([SUB, SUBS_PER_WIN, LO], BF16_DT, tag="ut")
            msgs_sb = win_p.tile([SUB, SUBS_PER_WIN, F_OUT], BF16_DT, tag="msgs")
            s_all = win_p.tile([SUB, SUBS_PER_WIN, WIN], BF16_DT, tag="s")
            nc.vector.tensor_tensor(
                out=s_all[:],
                in0=iota_sb[:, None, :].to_broadcast([SUB, SUBS_PER_WIN, WIN]),
                in1=rl_sb[:, :, None].to_broadcast([SUB, SUBS_PER_WIN, WIN]),
                op=mybir.AluOpType.is_equal)
            win_tiles[w] = (at_sb, ut_sb, msgs_sb, s_all)

        def stage_a(w, e_off, psz):
            base = w * WIN_E + e_off
            halves = (512, 512) if psz == 1024 else (psz,)
            ef_sb = stream_p.tile([RADIAL, psz], BF16_DT, tag=f"ef{psz}")
            nc.sync.dma_start(ef_sb[:], ef[:, base:base + psz])
            # Xrep[p=(ks,is), ci, e] = x_s[e, 8*ci + is] (host-gathered,
            # pair-blocked so each partition is one contiguous run; on the
            # sync queue so load issue never queues behind activations)
            xoff = 4 * base
            x_sb = stream_p.tile([128, 4, psz], BF16_DT, tag=f"x{psz}")
            nc.sync.dma_start(x_sb[:].rearrange("p c e -> p (c e)"),
                              x4[:, xoff:xoff + 4 * psz])

            h3d = dram_p.tile([HID, psz], BF16_DT, tag=f"h3d{psz}")
            h0 = 0
            for hsz in halves:
                hs = slice(h0, h0 + hsz)
                z1 = psum_mlp.tile([HID, 512], FP32, tag="z")
                nc.tensor.matmul(out=z1[:, :hsz], lhsT=w1_sb[:],
                                 rhs=ef_sb[:, hs],
                                 start=True, stop=True, skip_group_check=True)
                h1 = stream_p.tile([HID, 512], BF16_DT, tag="h1")
                nc.scalar.activation(h1[:, :hsz], z1[:, :hsz], ACT_FUNC)
                z2 = psum_mlp.tile([HID, 512], FP32, tag="z")
                nc.tensor.matmul(out=z2[:, :hsz], lhsT=w2_sb[:],
                                 rhs=h1[:, :hsz],
                                 start=True, stop=True, skip_group_check=True)
                h2 = stream_p.tile([HID, 512], BF16_DT, tag="h2")
                nc.scalar.activation(h2[:, :hsz], z2[:, :hsz], ACT_FUNC)
                z3 = psum_mlp.tile([HID, 512], FP32, tag="z")
                nc.tensor.matmul(out=z3[:, :hsz], lhsT=w3_sb[:],
                                 rhs=h2[:, :hsz],
                                 start=True, stop=True, skip_group_check=True)
                h3p = stream_p.tile([HID, 512], BF16_DT, tag="h3")
                nc.scalar.activation(h3p[:, :hsz], z3[:, :hsz], ACT_FUNC)
                nc.sync.dma_start(h3d[:, hs], h3p[:, :hsz])
                h0 += hsz

            # broadcast: h3mid[p=8ks+is, ck, e] = h3p[4ks + ck, e] (8x rep;
            # src outer dim 16 spreads descriptors over 16 SDMA engines)
            pool = bc_p if psz == 1024 else tail_p
            h3mid = pool.tile([128, 4, psz], BF16_DT, tag=f"h3bc{psz}")
            dst = h3mid[:].rearrange("p c e -> p (c e)")
            src2 = h3d[:].rearrange("(ks ck) e -> ks (ck e)", ks=16)
            src2 = src2[:, None, :].to_broadcast([16, 8, 4 * psz])
            nc.sync.dma_start(dst, src2)
            return (w, e_off, psz, x_sb, h3mid)

        def stage_b(stt):
            w, e_off, psz, x_sb, h3mid = stt
            at_sb, ut_sb, msgs_sb, s_all = win_tiles[w]
            halves = (512, 512) if psz == 1024 else (psz,)
            h0 = 0
            for hsz in halves:
                hs = slice(h0, h0 + hsz)
                a_all = (chunk_p if hsz == 512 else tail_p).tile(
                    [128, N_CHUNK, hsz], BF16_DT, tag=f"a{hsz}")
                nc.vector.tensor_tensor(
                    out=a_all[:].rearrange("p (ck ci) e -> p ck ci e", ck=4),
                    in0=h3mid[:, :, None, hs].to_broadcast([128, 4, 4, hsz]),
                    in1=x_sb[:, None, :, hs].to_broadcast([128, 4, 4, hsz]),
                    op=mybir.AluOpType.mult)
                u_ps = psum_u.tile([LO, 512], FP32, tag="u")
                for c in range(N_CHUNK):
                    nc.tensor.matmul(out=u_ps[:, :hsz],
                                     lhsT=wg_sb[:, c * LO:(c + 1) * LO],
                                     rhs=a_all[:, c, :],
                                     start=(c == 0), stop=(c == N_CHUNK - 1),
                                     skip_group_check=True)
                u_sb = stream_p.tile([LO, 512], BF16_DT, tag="usb")
                nc.scalar.copy(u_sb[:, :hsz], u_ps[:, :hsz])
                ut_ps = psum_ut.tile([128, 4, LO], BF16_DT, tag="utp")
                nsub = hsz // SUB
                for s in range(nsub):
                    nc.tensor.transpose(
                        out=ut_ps[:, s, :],
                        in_=u_sb[:, s * SUB:(s + 1) * SUB],
                        identity=ident_sb[:LO, :LO])
                st0 = (e_off + h0) // SUB
                nc.scalar.copy(ut_sb[:, st0:st0 + nsub, :], ut_ps[:, :nsub, :])
                h0 += hsz
            if psz == 1024:
                msgs_range(ut_sb, at_sb, msgs_sb, e_off // SUB,
                           (e_off + psz) // SUB)

        tiles = [(w, e_off, psz) for w in range(WINS_PER_CORE)
                 for (e_off, psz) in ((0, 1024), (1024, 1024), (2048, 128))]
        prev = None
        for (w, e_off, psz) in tiles:
            if e_off == 0:
                win_start(w)
            cur = stage_a(w, e_off, psz)
            if prev is not None:
                stage_b(prev)
                if prev[2] == 128:  # last pair of its window
                    pw = prev[0]
                    wt = win_tiles.pop(pw)
                    flush_window(pw, wt[1], wt[0], wt[2], wt[3])
            prev = cur
        stage_b(prev)
        wt = win_tiles.pop(prev[0])
        flush_window(prev[0], wt[1], wt[0], wt[2], wt[3])

    nc.compile()
    return nc


def _host_prep(node_feats, edge_attrs, edge_feats, senders, receivers,
               W1, W2, W3, Wgen):
    """Sort/shard edges by receiver window, build per-core input maps."""
    senders = np.asarray(senders).astype(np.int64)
    receivers = np.asarray(receivers).astype(np.int64)
    node_feats = np.asarray(node_feats, dtype=np.float32)
    edge_attrs = np.asarray(edge_attrs, dtype=np.float32)
    edge_feats = np.asarray(edge_feats, dtype=np.float32)

    n_win_total = N_CORES * WINS_PER_CORE  # 80
    win_id = receivers // WIN
    order = np.argsort(win_id, kind="stable")
    counts = np.bincount(win_id, minlength=n_win_total)
    assert counts.max() <= WIN_E, f"window overflow: {counts.max()} > {WIN_E}"
    starts = np.zeros(n_win_total + 1, np.int64)
    np.cumsum(counts, out=starts[1:])

    # slot arrays (padded); padding edges: ef=0, attr=0 -> msgs contribution 0
    E_TOT = N_CORES * E_CORE
    ef_s = np.zeros((E_TOT, RADIAL), np.float32)
    at_s = np.zeros((E_TOT, NSH), np.float32)
    rl_s = np.zeros(E_TOT, np.float32)
    sd_s = np.zeros(E_TOT, np.int64)

    slot_base = np.arange(n_win_total) * WIN_E
    # positions for real edges
    within = np.arange(len(order)) - starts[win_id[order]]
    slots = slot_base[win_id[order]] + within
    ef_s[slots] = edge_feats[order]
    at_s[slots] = edge_attrs[order] * np.float32(1.0 / np.sqrt(AVG_NUM_NEIGHBORS))
    rl_s[slots] = (receivers[order] % WIN).astype(np.float32)
    sd_s[slots] = senders[order]

    # host-side sender gather, replicated 4x along partitions (bf16)
    xg = node_feats[sd_s].astype(BF16)            # [E_TOT, 32]

    # weights with fan-in scales folded
    w1 = (W1 * (1.0 / np.sqrt(RADIAL))).astype(BF16)
    w2 = (W2 * (1.0 / np.sqrt(HID))).astype(BF16)
    w3 = (W3 * (1.0 / np.sqrt(HID))).astype(np.float32)  # permuted below, cast after
    w3p = np.ascontiguousarray(w3).astype(BF16)
    # p = 8*ks + is, chunk c = 4*ck + ci: wg[c][p, lo] =
    #   Wgen[4*ks + ck, l, o, 8*ci + is] / sqrt(HID*C)
    wgen = np.asarray(Wgen, dtype=np.float32) * np.float32(1.0 / np.sqrt(HID * C))
    p = np.arange(128)
    ks, is_ = p // 8, p % 8
    wgc = np.zeros((N_CHUNK, 128, NL, C), np.float32)
    for ck in range(4):
        for ci in range(4):
            wgc[4 * ck + ci] = wgen[4 * ks + ck][p, :, :, 8 * ci + is_].reshape(
                128, NL, C)
    # -> [128, 16*96]: chunk-major along free dim
    wgc = wgc.reshape(N_CHUNK, 128, LO).transpose(1, 0, 2).reshape(
        128, N_CHUNK * LO)
    wgc = wgc.astype(BF16)

    iota = np.broadcast_to(np.arange(128, dtype=np.float32),
                           (128, 128)).copy()

    in_maps = []
    for m in range(N_CORES):
        sl = slice(m * E_CORE, (m + 1) * E_CORE)
        ef_c = ef_s[sl]      # [E_CORE, 8]
        at_c = at_s[sl]      # [E_CORE, 9]
        rl_c = rl_s[sl]
        # attrs compact bf16, subtile-major (broadcast along c on the DVE)
        atc = np.ascontiguousarray(
            at_c.astype(BF16).reshape(N_ST, SUB, NSH).transpose(1, 0, 2).reshape(
                SUB, N_ST * NSH))
        # x4m[p=(ks,is), ci, e] = xg[e, 8*ci + is], ks-replicated 16x,
        # then pair-blocked: per (window, pair) a contiguous [4*psz] run
        t = xg[sl].T.reshape(4, 8, E_CORE).transpose(1, 0, 2)  # [is, ci, e]
        x4f = np.broadcast_to(t[None], (16, 8, 4, E_CORE)).reshape(
            128, 4, E_CORE)
        blocks = []
        for wi in range(WINS_PER_CORE):
            for eo, psz in ((0, 1024), (1024, 1024), (2048, 128)):
                b = wi * WIN_E + eo
                blocks.append(x4f[:, :, b:b + psz].reshape(128, 4 * psz))
        x4_c = np.ascontiguousarray(np.concatenate(blocks, axis=1))
        in_maps.append({
            "ef": np.ascontiguousarray(ef_c.T.astype(BF16)),
            "at": atc,
            "rl": np.ascontiguousarray(
                rl_c.reshape(N_ST, SUB).T),
            "x4": x4_c,
            "w1": w1, "w2": w2, "w3": w3p, "wg": wgc,
            "iota": iota,
        })
    return in_maps


def kernel(node_feats, edge_attrs, edge_feats, senders, receivers,
           W1, W2, W3, Wgen):
    in_maps = _host_prep(node_feats, edge_attrs, edge_feats, senders, receivers,
                         W1, W2, W3, Wgen)
    if "nc" not in _CACHED:
        _CACHED["nc"] = _build_nc()
    nc = _CACHED["nc"]
    res = run_bass_kernel_spmd(nc, in_maps, core_ids=list(range(N_CORES)))
    outs = [res.results[m]["out"] for m in range(N_CORES)]
    full = np.concatenate(outs, axis=0)[:N_NODES]          # [10000, 288]
    out = full.reshape(N_NODES, NSH, C).transpose(0, 2, 1)  # [10000, 32, 9]
    return np.ascontiguousarray(out.astype(np.float32))


# revision 49
# speedup vs baseline: 1.0155x; 1.0155x over previous
"""MessagePassingConvolution kernel for 8 Trainium2 NeuronCores.

Strategy (no collectives; ~347us profiled on HW):
  - Host: sort edges by receiver; shard by receiver windows. Core m owns
    nodes [m*1280, (m+1)*1280) = 10 windows of 128 nodes. Each window's
    edge list is padded to a fixed budget (2176 = 17 subtiles of 128) so
    the SPMD program is identical across cores. The sender gather
    (node_feats[senders]) is done host-side.
  - Einsum u[e,lo] = sum_{k,i} h[e,k] x[e,i] Wgen[k,lo,i] runs as 16
    chunk matmuls over a materialized outer product A[e,(k,i)].  The
    (k,i)->partition/chunk mapping is chosen to minimize operand
    replication DMA: k = 4*ks + ck, i = 8*ci + is, partition p = 8ks+is,
    chunk (ck,ci).  h3 is replicated 8x (DRAM-bounce broadcast, 1KB/edge)
    and x 16x (free: baked into the host-gathered stream); the DVE builds
    A in one bf16 2x-mode op per 512 edges with both factors read via
    free-dim broadcasts.  The broadcast DMA's source APs put a >=16-size
    dim outermost, which is what makes the DGE spread descriptors over
    all 16 SDMA engines (outer dim 4 -> only 4 engines, 4x slower).
  - All matmuls bf16 (fp32 matmul runs LOW/HIGH double passes, ~3x cost).
  - Per (window, 1024-edge pair), software-pipelined with a 1-pair skew
    (stage A: loads + MLP + bounce; stage B: outer product + einsum +
    transposes) so load chains complete while the previous pair computes:
      MLP (3 bf16 matmuls + Silu) -> h3p [64, e] bf16
      h3mid[p, ck, e] = h3p[4ks+ck, e] via one broadcast DMA per pair
      A = h3mid*x (DVE), u[96,*] += Wg_c.T @ A_c (16 matmuls/512)
      tensor-engine transpose -> ut (bf16), msgs = ut * attrs (DVE)
      scatter: psum[128n, 288] += S.T @ msgs, S built on-device from
        recv_local via iota==scalar compare (one DVE op per window)
    The window's tail-subtile msgs + scatter + store are deferred into
    the next window so the DVE never stalls on the tail chain.
  - Output: per-core [1280, 288] slices -> concat -> [10000, 32, 9].
"""

import sys
import numpy as np
from contextlib import ExitStack

sys.path.insert(0, "/opt/trn_rl_repo")

import concourse.bass as bass  # noqa: E402
import concourse.bacc as bacc  # noqa: E402
import concourse.mybir as mybir  # noqa: E402
import concourse.tile as tile  # noqa: E402
from concourse.masks import make_identity  # noqa: E402
from concourse.bass_utils import run_bass_kernel_spmd  # noqa: E402

import ml_dtypes  # noqa: E402

BF16 = ml_dtypes.bfloat16

# ---- problem constants (hardcoded per spec) ----
N_NODES = 10000
N_EDGES = 160000
C = 32
RADIAL = 8
HID = 64
NL = 3
L_DIMS = (1, 3, 5)
NSH = 9  # sum(L_DIMS)
AVG_NUM_NEIGHBORS = 16.0

N_CORES = 8
WIN = 128                      # nodes per window (psum partitions)
WINS_PER_CORE = 10
NODES_PER_CORE = WIN * WINS_PER_CORE     # 1280
N_NODES_PAD = NODES_PER_CORE * N_CORES   # 10240
SUB = 128                      # edges per subtile
SUBS_PER_WIN = 17              # window edge budget = 2176 (data max 2155)
WIN_E = SUB * SUBS_PER_WIN     # 2176
E_CORE = WIN_E * WINS_PER_CORE  # 21760
N_ST = E_CORE // SUB           # 170 subtiles per core
TILE_SIZES = (512, 512, 512, 512, 128)   # per-window einsum tiles
N_CHUNK = 16                   # ki chunks (2048 / 128)
LO = NL * C                    # 96
F_OUT = NSH * C                # 288

FP32 = mybir.dt.float32
BF16_DT = mybir.dt.bfloat16

_CACHED = {}

# CoreSim doesn't implement Silu; sim_test.py overrides this to Sigmoid and
# checks against a sigmoid-variant reference to validate the data plumbing.
ACT_FUNC = mybir.ActivationFunctionType.Silu


def _build_nc():
    nc = bacc.Bacc()

    ef = nc.dram_tensor("ef", [RADIAL, E_CORE], BF16_DT, kind="ExternalInput")
    x4 = nc.dram_tensor("x4", [128, 4 * E_CORE], BF16_DT, kind="ExternalInput")
    at = nc.dram_tensor("at", [SUB, N_ST * NSH], BF16_DT,
                        kind="ExternalInput")
    rl = nc.dram_tensor("rl", [SUB, N_ST], FP32, kind="ExternalInput")
    w1 = nc.dram_tensor("w1", [RADIAL, HID], BF16_DT, kind="ExternalInput")
    w2 = nc.dram_tensor("w2", [HID, HID], BF16_DT, kind="ExternalInput")
    w3 = nc.dram_tensor("w3", [HID, HID], BF16_DT, kind="ExternalInput")
    wg = nc.dram_tensor("wg", [128, N_CHUNK * LO], BF16_DT, kind="ExternalInput")
    iota = nc.dram_tensor("iota", [128, 128], FP32, kind="ExternalInput")
    out = nc.dram_tensor("out", [NODES_PER_CORE, F_OUT], FP32, kind="ExternalOutput")

    with tile.TileContext(nc) as tc, ExitStack() as ctx:
        const_p = ctx.enter_context(tc.tile_pool(name="const", bufs=1))
        stream_p = ctx.enter_context(tc.tile_pool(name="stream", bufs=3))
        win_p = ctx.enter_context(tc.tile_pool(name="win", bufs=2))
        chunk_p = ctx.enter_context(tc.tile_pool(name="chunk", bufs=3))
        bc_p = ctx.enter_context(tc.tile_pool(name="bc", bufs=3))
        psum_mlp = ctx.enter_context(tc.tile_pool(name="pmlp", bufs=2, space="PSUM"))
        psum_u = ctx.enter_context(tc.tile_pool(name="pu", bufs=3, space="PSUM"))
        psum_ut = ctx.enter_context(tc.tile_pool(name="put", bufs=2, space="PSUM"))
        psum_acc = ctx.enter_context(tc.tile_pool(name="pacc", bufs=1, space="PSUM"))
        dram_p = ctx.enter_context(tc.tile_pool(name="dram", bufs=3, space="DRAM"))
        tail_p = ctx.enter_context(tc.tile_pool(name="tail", bufs=1))

        # ---- one-time constants into SBUF ----
        w1_sb = const_p.tile([RADIAL, HID], BF16_DT)
        nc.scalar.dma_start(w1_sb[:], w1[:])
        w2_sb = const_p.tile([HID, HID], BF16_DT)
        nc.scalar.dma_start(w2_sb[:], w2[:])
        w3_sb = const_p.tile([HID, HID], BF16_DT)
        nc.scalar.dma_start(w3_sb[:], w3[:])
        wg_sb = const_p.tile([128, N_CHUNK * LO], BF16_DT)
        nc.scalar.dma_start(wg_sb[:], wg[:])
        iota_sb = const_p.tile([128, 128], FP32)
        nc.scalar.dma_start(iota_sb[:], iota[:])
        ident_sb = const_p.tile([128, 128], BF16_DT)
        make_identity(nc, ident_sb[:])

        lofs = (0, 1, 4)

        def msgs_range(f_ut, f_at, f_msgs, s0, s1):
            # msgs = ut * attr for subtiles [s0, s1) (attr bcast along c)
            ns = s1 - s0
            for l in range(NL):
                dim = L_DIMS[l]
                u_ap = f_ut[:, s0:s1, None, l * C:(l + 1) * C].to_broadcast(
                    [SUB, ns, dim, C])
                a_ap = f_at[:].rearrange("p (s m) -> p s m", m=NSH)[
                    :, s0:s1, lofs[l]:lofs[l] + dim]
                a_ap = a_ap[:, :, :, None].to_broadcast([SUB, ns, dim, C])
                nc.vector.tensor_tensor(
                    out=f_msgs[:, s0:s1, lofs[l] * C:(lofs[l] + dim) * C]
                    .rearrange("p s (m c) -> p s m c", c=C),
                    in0=u_ap, in1=a_ap, op=mybir.AluOpType.mult)

        def flush_window(fw, f_ut, f_at, f_msgs, f_s):
            # tail subtile's msgs, then scatter + store
            msgs_range(f_ut, f_at, f_msgs, SUBS_PER_WIN - 1, SUBS_PER_WIN)
            # scatter: psum_out += S.T @ msgs per subtile
            acc = psum_acc.tile([WIN, F_OUT], FP32, tag="acc")
            for st in range(SUBS_PER_WIN):
                nc.tensor.matmul(out=acc[:], lhsT=f_s[:, st, :],
                                 rhs=f_msgs[:, st, :],
                                 start=(st == 0), stop=(st == SUBS_PER_WIN - 1),
                                 skip_group_check=True)
            out_sb = stream_p.tile([WIN, F_OUT], FP32, tag="osb")
            nc.scalar.copy(out_sb[:], acc[:])
            nc.scalar.dma_start(out[fw * WIN:(fw + 1) * WIN, :], out_sb[:])

        # ---- flat skewed pipeline over all (window, pair) tiles ----
        # stage A issues DMAs + MLP + bounce for a pair; stage B (one step
        # later) does the outer product + einsum + transposes, so each
        # pair's load chain completes while the previous pair computes.
        win_tiles = {}

        def win_start(w):
            at_sb = win_p.tile([SUB, SUBS_PER_WIN * NSH], BF16_DT, tag="at")
            nc.scalar.dma_start(
                at_sb[:],
                at[:, w * SUBS_PER_WIN * NSH:(w + 1) * SUBS_PER_WIN * NSH])
            rl_sb = win_p.tile([SUB, SUBS_PER_WIN], FP32, tag="rl")
            nc.scalar.dma_start(
                rl_sb[:], rl[:, w * SUBS_PER_WIN:(w + 1) * SUBS_PER_WIN])
            ut_sb = win_p.tile([SUB, SUBS_PER_WIN, LO], BF16_DT, tag="ut")
            msgs_sb = win_p.tile([SUB, SUBS_PER_WIN, F_OUT], BF16_DT, tag="msgs")
            s_all = win_p.tile([SUB, SUBS_PER_WIN, WIN], BF16_DT, tag="s")
            nc.vector.tensor_tensor(
                out=s_all[:],
                in0=iota_sb[:, None, :].to_broadcast([SUB, SUBS_PER_WIN, WIN]),
                in1=rl_sb[:, :, None].to_broadcast([SUB, SUBS_PER_WIN, WIN]),
                op=mybir.AluOpType.is_equal)
            win_tiles[w] = (at_sb, ut_sb, msgs_sb, s_all)

        def stage_a(w, e_off, psz):
            base = w * WIN_E + e_off
            halves = (512, 512) if psz == 1024 else (psz,)
            ef_sb = stream_p.tile([RADIAL, psz], BF16_DT, tag=f"ef{psz}")
            nc.sync.dma_start(ef_sb[:], ef[:, base:base + psz])
            # Xrep[p=(ks,is), ci, e] = x_s[e, 8*ci + is] (host-gathered,
            # pair-blocked so each partition is one contiguous run; on the
            # sync queue so load issue never queues behind activations)
            xoff = 4 * base
            x_sb = stream_p.tile([128, 4, psz], BF16_DT, tag=f"x{psz}")
            nc.sync.dma_start(x_sb[:].rearrange("p c e -> p (c e)"),
                              x4[:, xoff:xoff + 4 * psz])

            h3d = dram_p.tile([HID, psz], BF16_DT, tag=f"h3d{psz}")
            h0 = 0
            for hsz in halves:
                hs = slice(h0, h0 + hsz)
                z1 = psum_mlp.tile([HID, 512], FP32, tag="z")
                nc.tensor.matmul(out=z1[:, :hsz], lhsT=w1_sb[:],
                                 rhs=ef_sb[:, hs],
                                 start=True, stop=True, skip_group_check=True)
                h1 = stream_p.tile([HID, 512], BF16_DT, tag="h1")
                nc.scalar.activation(h1[:, :hsz], z1[:, :hsz], ACT_FUNC)
                z2 = psum_mlp.tile([HID, 512], FP32, tag="z")
                nc.tensor.matmul(out=z2[:, :hsz], lhsT=w2_sb[:],
                                 rhs=h1[:, :hsz],
                                 start=True, stop=True, skip_group_check=True)
                h2 = stream_p.tile([HID, 512], BF16_DT, tag="h2")
                nc.scalar.activation(h2[:, :hsz], z2[:, :hsz], ACT_FUNC)
                z3 = psum_mlp.tile([HID, 512], FP32, tag="z")
                nc.tensor.matmul(out=z3[:, :hsz], lhsT=w3_sb[:],
                                 rhs=h2[:, :hsz],
                                 start=True, stop=True, skip_group_check=True)
                h3p = stream_p.tile([HID, 512], BF16_DT, tag="h3")
                nc.scalar.activation(h3p[:, :hsz], z3[:, :hsz], ACT_FUNC)
                nc.sync.dma_start(h3d[:, hs], h3p[:, :hsz])
                h0 += hsz

            # broadcast: h3mid[p=8ks+is, ck, e] = h3p[4ks + ck, e] (8x rep;
            # src outer dim 16 spreads descriptors over 16 SDMA engines)
            pool = bc_p if psz == 1024 else tail_p
            h3mid = pool.tile([128, 4, psz], BF16_DT, tag=f"h3bc{psz}")
            dst = h3mid[:].rearrange("p c e -> p (c e)")
            src2 = h3d[:].rearrange("(ks ck) e -> ks (ck e)", ks=16)
            src2 = src2[:, None, :].to_broadcast([16, 8, 4 * psz])
            nc.sync.dma_start(dst, src2)
            return (w, e_off, psz, x_sb, h3mid)

        def stage_b(stt):
            w, e_off, psz, x_sb, h3mid = stt
            at_sb, ut_sb, msgs_sb, s_all = win_tiles[w]
            halves = (512, 512) if psz == 1024 else (psz,)
            h0 = 0
            for hsz in halves:
                hs = slice(h0, h0 + hsz)
                a_all = (chunk_p if hsz == 512 else tail_p).tile(
                    [128, N_CHUNK, hsz], BF16_DT, tag=f"a{hsz}")
                nc.vector.tensor_tensor(
                    out=a_all[:].rearrange("p (ck ci) e -> p ck ci e", ck=4),
                    in0=h3mid[:, :, None, hs].to_broadcast([128, 4, 4, hsz]),
                    in1=x_sb[:, None, :, hs].to_broadcast([128, 4, 4, hsz]),
                    op=mybir.AluOpType.mult)
                u_ps = psum_u.tile([LO, 512], FP32, tag="u")
                for c in range(N_CHUNK):
                    nc.tensor.matmul(out=u_ps[:, :hsz],
                                     lhsT=wg_sb[:, c * LO:(c + 1) * LO],
                                     rhs=a_all[:, c, :],
                                     start=(c == 0), stop=(c == N_CHUNK - 1),
                                     skip_group_check=True)
                u_sb = stream_p.tile([LO, 512], BF16_DT, tag="usb")
                nc.scalar.copy(u_sb[:, :hsz], u_ps[:, :hsz])
                ut_ps = psum_ut.tile([128, 4, LO], BF16_DT, tag="utp")
                nsub = hsz // SUB
                for s in range(nsub):
                    nc.tensor.transpose(
                        out=ut_ps[:, s, :],
                        in_=u_sb[:, s * SUB:(s + 1) * SUB],
                        identity=ident_sb[:LO, :LO])
                st0 = (e_off + h0) // SUB
                nc.scalar.copy(ut_sb[:, st0:st0 + nsub, :], ut_ps[:, :nsub, :])
                h0 += hsz
            if psz == 1024:
                msgs_range(ut_sb, at_sb, msgs_sb, e_off // SUB,
                           (e_off + psz) // SUB)

        tiles = [(w, e_off, psz) for w in range(WINS_PER_CORE)
                 for (e_off, psz) in ((0, 1024), (1024, 1024), (2048, 128))]
        SKEW = 2
        inflight = []
        for (w, e_off, psz) in tiles:
            if e_off == 0:
                win_start(w)
            inflight.append(stage_a(w, e_off, psz))
            if len(inflight) > SKEW:
                prev = inflight.pop(0)
                stage_b(prev)
                if prev[2] == 128:  # last pair of its window
                    pw = prev[0]
                    wt = win_tiles.pop(pw)
                    flush_window(pw, wt[1], wt[0], wt[2], wt[3])
        for prev in inflight:
            stage_b(prev)
            if prev[2] == 128:
                pw = prev[0]
                wt = win_tiles.pop(pw)
                flush_window(pw, wt[1], wt[0], wt[2], wt[3])

    nc.compile()
    return nc


def _host_prep(node_feats, edge_attrs, edge_feats, senders, receivers,
               W1, W2, W3, Wgen):
    """Sort/shard edges by receiver window, build per-core input maps."""
    senders = np.asarray(senders).astype(np.int64)
    receivers = np.asarray(receivers).astype(np.int64)
    node_feats = np.asarray(node_feats, dtype=np.float32)
    edge_attrs = np.asarray(edge_attrs, dtype=np.float32)
    edge_feats = np.asarray(edge_feats, dtype=np.float32)

    n_win_total = N_CORES * WINS_PER_CORE  # 80
    win_id = receivers // WIN
    order = np.argsort(win_id, kind="stable")
    counts = np.bincount(win_id, minlength=n_win_total)
    assert counts.max() <= WIN_E, f"window overflow: {counts.max()} > {WIN_E}"
    starts = np.zeros(n_win_total + 1, np.int64)
    np.cumsum(counts, out=starts[1:])

    # slot arrays (padded); padding edges: ef=0, attr=0 -> msgs contribution 0
    E_TOT = N_CORES * E_CORE
    ef_s = np.zeros((E_TOT, RADIAL), np.float32)
    at_s = np.zeros((E_TOT, NSH), np.float32)
    rl_s = np.zeros(E_TOT, np.float32)
    sd_s = np.zeros(E_TOT, np.int64)

    slot_base = np.arange(n_win_total) * WIN_E
    # positions for real edges
    within = np.arange(len(order)) - starts[win_id[order]]
    slots = slot_base[win_id[order]] + within
    ef_s[slots] = edge_feats[order]
    at_s[slots] = edge_attrs[order] * np.float32(1.0 / np.sqrt(AVG_NUM_NEIGHBORS))
    rl_s[slots] = (receivers[order] % WIN).astype(np.float32)
    sd_s[slots] = senders[order]

    # host-side sender gather, replicated 4x along partitions (bf16)
    xg = node_feats[sd_s].astype(BF16)            # [E_TOT, 32]

    # weights with fan-in scales folded
    w1 = (W1 * (1.0 / np.sqrt(RADIAL))).astype(BF16)
    w2 = (W2 * (1.0 / np.sqrt(HID))).astype(BF16)
    w3 = (W3 * (1.0 / np.sqrt(HID))).astype(np.float32)  # permuted below, cast after
    w3p = np.ascontiguousarray(w3).astype(BF16)
    # p = 8*ks + is, chunk c = 4*ck + ci: wg[c][p, lo] =
    #   Wgen[4*ks + ck, l, o, 8*ci + is] / sqrt(HID*C)
    wgen = np.asarray(Wgen, dtype=np.float32) * np.float32(1.0 / np.sqrt(HID * C))
    p = np.arange(128)
    ks, is_ = p // 8, p % 8
    wgc = np.zeros((N_CHUNK, 128, NL, C), np.float32)
    for ck in range(4):
        for ci in range(4):
            wgc[4 * ck + ci] = wgen[4 * ks + ck][p, :, :, 8 * ci + is_].reshape(
                128, NL, C)
    # -> [128, 16*96]: chunk-major along free dim
    wgc = wgc.reshape(N_CHUNK, 128, LO).transpose(1, 0, 2).reshape(
        128, N_CHUNK * LO)
    wgc = wgc.astype(BF16)

    iota = np.broadcast_to(np.arange(128, dtype=np.float32),
                           (128, 128)).copy()

    in_maps = []
    for m in range(N_CORES):
        sl = slice(m * E_CORE, (m + 1) * E_CORE)
        ef_c = ef_s[sl]      # [E_CORE, 8]
        at_c = at_s[sl]      # [E_CORE, 9]
        rl_c = rl_s[sl]
        # attrs compact bf16, subtile-major (broadcast along c on the DVE)
        atc = np.ascontiguousarray(
            at_c.astype(BF16).reshape(N_ST, SUB, NSH).transpose(1, 0, 2).reshape(
                SUB, N_ST * NSH))
        # x4m[p=(ks,is), ci, e] = xg[e, 8*ci + is], ks-replicated 16x,
        # then pair-blocked: per (window, pair) a contiguous [4*psz] run
        t = xg[sl].T.reshape(4, 8, E_CORE).transpose(1, 0, 2)  # [is, ci, e]
        x4f = np.broadcast_to(t[None], (16, 8, 4, E_CORE)).reshape(
            128, 4, E_CORE)
        blocks = []
        for wi in range(WINS_PER_CORE):
            for eo, psz in ((0, 1024), (1024, 1024), (2048, 128)):
                b = wi * WIN_E + eo
                blocks.append(x4f[:, :, b:b + psz].reshape(128, 4 * psz))
        x4_c = np.ascontiguousarray(np.concatenate(blocks, axis=1))
        in_maps.append({
            "ef": np.ascontiguousarray(ef_c.T.astype(BF16)),
            "at": atc,
            "rl": np.ascontiguousarray(
                rl_c.reshape(N_ST, SUB).T),
            "x4": x4_c,
            "w1": w1, "w2": w2, "w3": w3p, "wg": wgc,
            "iota": iota,
        })
    return in_maps


def kernel(node_feats, edge_attrs, edge_feats, senders, receivers,
           W1, W2, W3, Wgen):
    in_maps = _host_prep(node_feats, edge_attrs, edge_feats, senders, receivers,
                         W1, W2, W3, Wgen)
    if "nc" not in _CACHED:
        _CACHED["nc"] = _build_nc()
    nc = _CACHED["nc"]
    res = run_bass_kernel_spmd(nc, in_maps, core_ids=list(range(N_CORES)))
    outs = [res.results[m]["out"] for m in range(N_CORES)]
    full = np.concatenate(outs, axis=0)[:N_NODES]          # [10000, 288]
    out = full.reshape(N_NODES, NSH, C).transpose(0, 2, 1)  # [10000, 32, 9]
    return np.ascontiguousarray(out.astype(np.float32))


# revision 50
# speedup vs baseline: 1.1298x; 1.1125x over previous
"""MessagePassingConvolution kernel for 8 Trainium2 NeuronCores.

Strategy (no collectives; ~347us profiled on HW):
  - Host: sort edges by receiver; shard by receiver windows. Core m owns
    nodes [m*1280, (m+1)*1280) = 10 windows of 128 nodes. Each window's
    edge list is padded to a fixed budget (2176 = 17 subtiles of 128) so
    the SPMD program is identical across cores. The sender gather
    (node_feats[senders]) is done host-side.
  - Einsum u[e,lo] = sum_{k,i} h[e,k] x[e,i] Wgen[k,lo,i] runs as 16
    chunk matmuls over a materialized outer product A[e,(k,i)].  The
    (k,i)->partition/chunk mapping is chosen to minimize operand
    replication DMA: k = 4*ks + ck, i = 8*ci + is, partition p = 8ks+is,
    chunk (ck,ci).  h3 is replicated 8x (DRAM-bounce broadcast, 1KB/edge)
    and x 16x (free: baked into the host-gathered stream); the DVE builds
    A in one bf16 2x-mode op per 512 edges with both factors read via
    free-dim broadcasts.  The broadcast DMA's source APs put a >=16-size
    dim outermost, which is what makes the DGE spread descriptors over
    all 16 SDMA engines (outer dim 4 -> only 4 engines, 4x slower).
  - All matmuls bf16 (fp32 matmul runs LOW/HIGH double passes, ~3x cost).
  - Per (window, 1024-edge pair), software-pipelined with a 1-pair skew
    (stage A: loads + MLP + bounce; stage B: outer product + einsum +
    transposes) so load chains complete while the previous pair computes:
      MLP (3 bf16 matmuls + Silu) -> h3p [64, e] bf16
      h3mid[p, ck, e] = h3p[4ks+ck, e] via one broadcast DMA per pair
      A = h3mid*x (DVE), u[96,*] += Wg_c.T @ A_c (16 matmuls/512)
      tensor-engine transpose -> ut (bf16), msgs = ut * attrs (DVE)
      scatter: psum[128n, 288] += S.T @ msgs, S built on-device from
        recv_local via iota==scalar compare (one DVE op per window)
    The window's tail-subtile msgs + scatter + store are deferred into
    the next window so the DVE never stalls on the tail chain.
  - Output: per-core [1280, 288] slices -> concat -> [10000, 32, 9].
"""

import sys
import numpy as np
from contextlib import ExitStack

sys.path.insert(0, "/opt/trn_rl_repo")

import concourse.bass as bass  # noqa: E402
import concourse.bacc as bacc  # noqa: E402
import concourse.mybir as mybir  # noqa: E402
import concourse.tile as tile  # noqa: E402
from concourse.masks import make_identity  # noqa: E402
from concourse.bass_utils import run_bass_kernel_spmd  # noqa: E402

import ml_dtypes  # noqa: E402

BF16 = ml_dtypes.bfloat16

# ---- problem constants (hardcoded per spec) ----
N_NODES = 10000
N_EDGES = 160000
C = 32
RADIAL = 8
HID = 64
NL = 3
L_DIMS = (1, 3, 5)
NSH = 9  # sum(L_DIMS)
AVG_NUM_NEIGHBORS = 16.0

N_CORES = 8
WIN = 128                      # nodes per window (psum partitions)
WINS_PER_CORE = 10
NODES_PER_CORE = WIN * WINS_PER_CORE     # 1280
N_NODES_PAD = NODES_PER_CORE * N_CORES   # 10240
SUB = 128                      # edges per subtile
SUBS_PER_WIN = 17              # window edge budget = 2176 (data max 2155)
WIN_E = SUB * SUBS_PER_WIN     # 2176
E_CORE = WIN_E * WINS_PER_CORE  # 21760
N_ST = E_CORE // SUB           # 170 subtiles per core
TILE_SIZES = (512, 512, 512, 512, 128)   # per-window einsum tiles
N_CHUNK = 16                   # ki chunks (2048 / 128)
LO = NL * C                    # 96
F_OUT = NSH * C                # 288

FP32 = mybir.dt.float32
BF16_DT = mybir.dt.bfloat16

_CACHED = {}

# CoreSim doesn't implement Silu; sim_test.py overrides this to Sigmoid and
# checks against a sigmoid-variant reference to validate the data plumbing.
ACT_FUNC = mybir.ActivationFunctionType.Silu


def _build_nc():
    nc = bacc.Bacc()

    ef = nc.dram_tensor("ef", [RADIAL, E_CORE], BF16_DT, kind="ExternalInput")
    x4 = nc.dram_tensor("x4", [128, 4 * E_CORE], BF16_DT, kind="ExternalInput")
    at = nc.dram_tensor("at", [SUB, N_ST * NSH], BF16_DT,
                        kind="ExternalInput")
    rl = nc.dram_tensor("rl", [SUB, N_ST], FP32, kind="ExternalInput")
    w1 = nc.dram_tensor("w1", [RADIAL, HID], BF16_DT, kind="ExternalInput")
    w2 = nc.dram_tensor("w2", [HID, HID], BF16_DT, kind="ExternalInput")
    w3 = nc.dram_tensor("w3", [HID, HID], BF16_DT, kind="ExternalInput")
    wg = nc.dram_tensor("wg", [128, N_CHUNK * LO], BF16_DT, kind="ExternalInput")
    iota = nc.dram_tensor("iota", [128, 128], FP32, kind="ExternalInput")
    out = nc.dram_tensor("out", [NODES_PER_CORE, F_OUT], FP32, kind="ExternalOutput")

    with tile.TileContext(nc) as tc, ExitStack() as ctx:
        const_p = ctx.enter_context(tc.tile_pool(name="const", bufs=1))
        stream_p = ctx.enter_context(tc.tile_pool(name="stream", bufs=3))
        win_p = ctx.enter_context(tc.tile_pool(name="win", bufs=2))
        chunk_p = ctx.enter_context(tc.tile_pool(name="chunk", bufs=3))
        bc_p = ctx.enter_context(tc.tile_pool(name="bc", bufs=3))
        psum_mlp = ctx.enter_context(tc.tile_pool(name="pmlp", bufs=2, space="PSUM"))
        psum_u = ctx.enter_context(tc.tile_pool(name="pu", bufs=3, space="PSUM"))
        psum_ut = ctx.enter_context(tc.tile_pool(name="put", bufs=2, space="PSUM"))
        psum_acc = ctx.enter_context(tc.tile_pool(name="pacc", bufs=1, space="PSUM"))
        dram_p = ctx.enter_context(tc.tile_pool(name="dram", bufs=3, space="DRAM"))
        tail_p = ctx.enter_context(tc.tile_pool(name="tail", bufs=1))

        # ---- one-time constants into SBUF ----
        w1_sb = const_p.tile([RADIAL, HID], BF16_DT)
        nc.scalar.dma_start(w1_sb[:], w1[:])
        w2_sb = const_p.tile([2 * HID, HID], BF16_DT)
        nc.scalar.dma_start(w2_sb[:HID], w2[:])
        nc.scalar.dma_start(w2_sb[HID:], w2[:])
        w3_sb = const_p.tile([2 * HID, HID], BF16_DT)
        nc.scalar.dma_start(w3_sb[:HID], w3[:])
        nc.scalar.dma_start(w3_sb[HID:], w3[:])
        wg_sb = const_p.tile([128, N_CHUNK * LO], BF16_DT)
        nc.scalar.dma_start(wg_sb[:], wg[:])
        iota_sb = const_p.tile([128, 128], FP32)
        nc.scalar.dma_start(iota_sb[:], iota[:])
        ident_sb = const_p.tile([128, 128], BF16_DT)
        make_identity(nc, ident_sb[:])

        lofs = (0, 1, 4)

        def msgs_range(f_ut, f_at, f_msgs, s0, s1):
            # msgs = ut * attr for subtiles [s0, s1) (attr bcast along c)
            ns = s1 - s0
            for l in range(NL):
                dim = L_DIMS[l]
                u_ap = f_ut[:, s0:s1, None, l * C:(l + 1) * C].to_broadcast(
                    [SUB, ns, dim, C])
                a_ap = f_at[:].rearrange("p (s m) -> p s m", m=NSH)[
                    :, s0:s1, lofs[l]:lofs[l] + dim]
                a_ap = a_ap[:, :, :, None].to_broadcast([SUB, ns, dim, C])
                nc.vector.tensor_tensor(
                    out=f_msgs[:, s0:s1, lofs[l] * C:(lofs[l] + dim) * C]
                    .rearrange("p s (m c) -> p s m c", c=C),
                    in0=u_ap, in1=a_ap, op=mybir.AluOpType.mult)

        def flush_window(fw, f_ut, f_at, f_msgs, f_s):
            # tail subtile's msgs, then scatter + store
            msgs_range(f_ut, f_at, f_msgs, SUBS_PER_WIN - 1, SUBS_PER_WIN)
            # scatter: psum_out += S.T @ msgs per subtile
            acc = psum_acc.tile([WIN, F_OUT], FP32, tag="acc")
            for st in range(SUBS_PER_WIN):
                nc.tensor.matmul(out=acc[:], lhsT=f_s[:, st, :],
                                 rhs=f_msgs[:, st, :],
                                 start=(st == 0), stop=(st == SUBS_PER_WIN - 1),
                                 skip_group_check=True)
            out_sb = stream_p.tile([WIN, F_OUT], FP32, tag="osb")
            nc.scalar.copy(out_sb[:], acc[:])
            nc.scalar.dma_start(out[fw * WIN:(fw + 1) * WIN, :], out_sb[:])

        # ---- flat skewed pipeline over all (window, pair) tiles ----
        # stage A issues DMAs + MLP + bounce for a pair; stage B (one step
        # later) does the outer product + einsum + transposes, so each
        # pair's load chain completes while the previous pair computes.
        win_tiles = {}

        def win_start(w):
            at_sb = win_p.tile([SUB, SUBS_PER_WIN * NSH], BF16_DT, tag="at")
            nc.scalar.dma_start(
                at_sb[:],
                at[:, w * SUBS_PER_WIN * NSH:(w + 1) * SUBS_PER_WIN * NSH])
            rl_sb = win_p.tile([SUB, SUBS_PER_WIN], FP32, tag="rl")
            nc.scalar.dma_start(
                rl_sb[:], rl[:, w * SUBS_PER_WIN:(w + 1) * SUBS_PER_WIN])
            ut_sb = win_p.tile([SUB, SUBS_PER_WIN, LO], BF16_DT, tag="ut")
            msgs_sb = win_p.tile([SUB, SUBS_PER_WIN, F_OUT], BF16_DT, tag="msgs")
            s_all = win_p.tile([SUB, SUBS_PER_WIN, WIN], BF16_DT, tag="s")
            nc.vector.tensor_tensor(
                out=s_all[:],
                in0=iota_sb[:, None, :].to_broadcast([SUB, SUBS_PER_WIN, WIN]),
                in1=rl_sb[:, :, None].to_broadcast([SUB, SUBS_PER_WIN, WIN]),
                op=mybir.AluOpType.is_equal)
            win_tiles[w] = (at_sb, ut_sb, msgs_sb, s_all)

        def stage_a(w, e_off, psz):
            base = w * WIN_E + e_off
            halves = (512, 512) if psz == 1024 else (psz,)
            ef_sb = stream_p.tile([RADIAL, psz], BF16_DT, tag=f"ef{psz}")
            nc.sync.dma_start(ef_sb[:], ef[:, base:base + psz])
            # Xrep[p=(ks,is), ci, e] = x_s[e, 8*ci + is] (host-gathered,
            # pair-blocked so each partition is one contiguous run; on the
            # sync queue so load issue never queues behind activations)
            xoff = 4 * base
            x_sb = stream_p.tile([128, 4, psz], BF16_DT, tag=f"x{psz}")
            nc.sync.dma_start(x_sb[:].rearrange("p c e -> p (c e)"),
                              x4[:, xoff:xoff + 4 * psz])

            # MLP with the pair's two 512-halves stacked on partition
            # ranges [0:64] / [64:128]: one activation per layer per pair
            h3d = dram_p.tile([HID, psz], BF16_DT, tag=f"h3d{psz}")
            nh = len(halves)
            hw = halves[0]
            z1 = psum_mlp.tile([128, 512], FP32, tag="z")
            for i in range(nh):
                nc.tensor.matmul(out=z1[64 * i:64 * i + HID, :hw],
                                 lhsT=w1_sb[:],
                                 rhs=ef_sb[:, i * hw:(i + 1) * hw],
                                 start=True, stop=True, skip_group_check=True)
            h1 = stream_p.tile([128, 512], BF16_DT, tag="h1")
            nc.scalar.activation(h1[:64 * nh, :hw], z1[:64 * nh, :hw], ACT_FUNC)
            z2 = psum_mlp.tile([128, 512], FP32, tag="z")
            for i in range(nh):
                nc.tensor.matmul(out=z2[64 * i:64 * i + HID, :hw],
                                 lhsT=w2_sb[64 * i:64 * i + HID],
                                 rhs=h1[64 * i:64 * i + HID, :hw],
                                 start=True, stop=True, skip_group_check=True)
            h2 = stream_p.tile([128, 512], BF16_DT, tag="h2")
            nc.scalar.activation(h2[:64 * nh, :hw], z2[:64 * nh, :hw], ACT_FUNC)
            z3 = psum_mlp.tile([128, 512], FP32, tag="z")
            for i in range(nh):
                nc.tensor.matmul(out=z3[64 * i:64 * i + HID, :hw],
                                 lhsT=w3_sb[64 * i:64 * i + HID],
                                 rhs=h2[64 * i:64 * i + HID, :hw],
                                 start=True, stop=True, skip_group_check=True)
            h3p = stream_p.tile([128, 512], BF16_DT, tag="h3")
            nc.scalar.activation(h3p[:64 * nh, :hw], z3[:64 * nh, :hw], ACT_FUNC)
            for i in range(nh):
                nc.sync.dma_start(h3d[:, i * hw:(i + 1) * hw],
                                  h3p[64 * i:64 * i + HID, :hw])

            # broadcast: h3mid[p=8ks+is, ck, e] = h3p[4ks + ck, e] (8x rep;
            # src outer dim 16 spreads descriptors over 16 SDMA engines)
            pool = bc_p if psz == 1024 else tail_p
            h3mid = pool.tile([128, 4, psz], BF16_DT, tag=f"h3bc{psz}")
            dst = h3mid[:].rearrange("p c e -> p (c e)")
            src2 = h3d[:].rearrange("(ks ck) e -> ks (ck e)", ks=16)
            src2 = src2[:, None, :].to_broadcast([16, 8, 4 * psz])
            nc.sync.dma_start(dst, src2)
            return (w, e_off, psz, x_sb, h3mid)

        def stage_b(stt):
            w, e_off, psz, x_sb, h3mid = stt
            at_sb, ut_sb, msgs_sb, s_all = win_tiles[w]
            halves = (512, 512) if psz == 1024 else (psz,)
            h0 = 0
            for hsz in halves:
                hs = slice(h0, h0 + hsz)
                a_all = (chunk_p if hsz == 512 else tail_p).tile(
                    [128, N_CHUNK, hsz], BF16_DT, tag=f"a{hsz}")
                nc.vector.tensor_tensor(
                    out=a_all[:].rearrange("p (ck ci) e -> p ck ci e", ck=4),
                    in0=h3mid[:, :, None, hs].to_broadcast([128, 4, 4, hsz]),
                    in1=x_sb[:, None, :, hs].to_broadcast([128, 4, 4, hsz]),
                    op=mybir.AluOpType.mult)
                u_ps = psum_u.tile([LO, 512], FP32, tag="u")
                for c in range(N_CHUNK):
                    nc.tensor.matmul(out=u_ps[:, :hsz],
                                     lhsT=wg_sb[:, c * LO:(c + 1) * LO],
                                     rhs=a_all[:, c, :],
                                     start=(c == 0), stop=(c == N_CHUNK - 1),
                                     skip_group_check=True)
                u_sb = stream_p.tile([LO, 512], BF16_DT, tag="usb")
                nc.scalar.copy(u_sb[:, :hsz], u_ps[:, :hsz])
                ut_ps = psum_ut.tile([128, 4, LO], BF16_DT, tag="utp")
                nsub = hsz // SUB
                for s in range(nsub):
                    nc.tensor.transpose(
                        out=ut_ps[:, s, :],
                        in_=u_sb[:, s * SUB:(s + 1) * SUB],
                        identity=ident_sb[:LO, :LO])
                st0 = (e_off + h0) // SUB
                nc.scalar.copy(ut_sb[:, st0:st0 + nsub, :], ut_ps[:, :nsub, :])
                h0 += hsz
            if psz == 1024:
                msgs_range(ut_sb, at_sb, msgs_sb, e_off // SUB,
                           (e_off + psz) // SUB)

        tiles = [(w, e_off, psz) for w in range(WINS_PER_CORE)
                 for (e_off, psz) in ((0, 1024), (1024, 1024), (2048, 128))]
        SKEW = 2
        inflight = []
        for (w, e_off, psz) in tiles:
            if e_off == 0:
                win_start(w)
            inflight.append(stage_a(w, e_off, psz))
            if len(inflight) > SKEW:
                prev = inflight.pop(0)
                stage_b(prev)
                if prev[2] == 128:  # last pair of its window
                    pw = prev[0]
                    wt = win_tiles.pop(pw)
                    flush_window(pw, wt[1], wt[0], wt[2], wt[3])
        for prev in inflight:
            stage_b(prev)
            if prev[2] == 128:
                pw = prev[0]
                wt = win_tiles.pop(pw)
                flush_window(pw, wt[1], wt[0], wt[2], wt[3])

    nc.compile()
    return nc


def _host_prep(node_feats, edge_attrs, edge_feats, senders, receivers,
               W1, W2, W3, Wgen):
    """Sort/shard edges by receiver window, build per-core input maps."""
    senders = np.asarray(senders).astype(np.int64)
    receivers = np.asarray(receivers).astype(np.int64)
    node_feats = np.asarray(node_feats, dtype=np.float32)
    edge_attrs = np.asarray(edge_attrs, dtype=np.float32)
    edge_feats = np.asarray(edge_feats, dtype=np.float32)

    n_win_total = N_CORES * WINS_PER_CORE  # 80
    win_id = receivers // WIN
    order = np.argsort(win_id, kind="stable")
    counts = np.bincount(win_id, minlength=n_win_total)
    assert counts.max() <= WIN_E, f"window overflow: {counts.max()} > {WIN_E}"
    starts = np.zeros(n_win_total + 1, np.int64)
    np.cumsum(counts, out=starts[1:])

    # slot arrays (padded); padding edges: ef=0, attr=0 -> msgs contribution 0
    E_TOT = N_CORES * E_CORE
    ef_s = np.zeros((E_TOT, RADIAL), np.float32)
    at_s = np.zeros((E_TOT, NSH), np.float32)
    rl_s = np.zeros(E_TOT, np.float32)
    sd_s = np.zeros(E_TOT, np.int64)

    slot_base = np.arange(n_win_total) * WIN_E
    # positions for real edges
    within = np.arange(len(order)) - starts[win_id[order]]
    slots = slot_base[win_id[order]] + within
    ef_s[slots] = edge_feats[order]
    at_s[slots] = edge_attrs[order] * np.float32(1.0 / np.sqrt(AVG_NUM_NEIGHBORS))
    rl_s[slots] = (receivers[order] % WIN).astype(np.float32)
    sd_s[slots] = senders[order]

    # host-side sender gather, replicated 4x along partitions (bf16)
    xg = node_feats[sd_s].astype(BF16)            # [E_TOT, 32]

    # weights with fan-in scales folded
    w1 = (W1 * (1.0 / np.sqrt(RADIAL))).astype(BF16)
    w2 = (W2 * (1.0 / np.sqrt(HID))).astype(BF16)
    w3 = (W3 * (1.0 / np.sqrt(HID))).astype(np.float32)  # permuted below, cast after
    w3p = np.ascontiguousarray(w3).astype(BF16)
    # p = 8*ks + is, chunk c = 4*ck + ci: wg[c][p, lo] =
    #   Wgen[4*ks + ck, l, o, 8*ci + is] / sqrt(HID*C)
    wgen = np.asarray(Wgen, dtype=np.float32) * np.float32(1.0 / np.sqrt(HID * C))
    p = np.arange(128)
    ks, is_ = p // 8, p % 8
    wgc = np.zeros((N_CHUNK, 128, NL, C), np.float32)
    for ck in range(4):
        for ci in range(4):
            wgc[4 * ck + ci] = wgen[4 * ks + ck][p, :, :, 8 * ci + is_].reshape(
                128, NL, C)
    # -> [128, 16*96]: chunk-major along free dim
    wgc = wgc.reshape(N_CHUNK, 128, LO).transpose(1, 0, 2).reshape(
        128, N_CHUNK * LO)
    wgc = wgc.astype(BF16)

    iota = np.broadcast_to(np.arange(128, dtype=np.float32),
                           (128, 128)).copy()

    in_maps = []
    for m in range(N_CORES):
        sl = slice(m * E_CORE, (m + 1) * E_CORE)
        ef_c = ef_s[sl]      # [E_CORE, 8]
        at_c = at_s[sl]      # [E_CORE, 9]
        rl_c = rl_s[sl]
        # attrs compact bf16, subtile-major (broadcast along c on the DVE)
        atc = np.ascontiguousarray(
            at_c.astype(BF16).reshape(N_ST, SUB, NSH).transpose(1, 0, 2).reshape(
                SUB, N_ST * NSH))
        # x4m[p=(ks,is), ci, e] = xg[e, 8*ci + is], ks-replicated 16x,
        # then pair-blocked: per (window, pair) a contiguous [4*psz] run
        t = xg[sl].T.reshape(4, 8, E_CORE).transpose(1, 0, 2)  # [is, ci, e]
        x4f = np.broadcast_to(t[None], (16, 8, 4, E_CORE)).reshape(
            128, 4, E_CORE)
        blocks = []
        for wi in range(WINS_PER_CORE):
            for eo, psz in ((0, 1024), (1024, 1024), (2048, 128)):
                b = wi * WIN_E + eo
                blocks.append(x4f[:, :, b:b + psz].reshape(128, 4 * psz))
        x4_c = np.ascontiguousarray(np.concatenate(blocks, axis=1))
        in_maps.append({
            "ef": np.ascontiguousarray(ef_c.T.astype(BF16)),
            "at": atc,
            "rl": np.ascontiguousarray(
                rl_c.reshape(N_ST, SUB).T),
            "x4": x4_c,
            "w1": w1, "w2": w2, "w3": w3p, "wg": wgc,
            "iota": iota,
        })
    return in_maps


def kernel(node_feats, edge_attrs, edge_feats, senders, receivers,
           W1, W2, W3, Wgen):
    in_maps = _host_prep(node_feats, edge_attrs, edge_feats, senders, receivers,
                         W1, W2, W3, Wgen)
    if "nc" not in _CACHED:
        _CACHED["nc"] = _build_nc()
    nc = _CACHED["nc"]
    res = run_bass_kernel_spmd(nc, in_maps, core_ids=list(range(N_CORES)))
    outs = [res.results[m]["out"] for m in range(N_CORES)]
    full = np.concatenate(outs, axis=0)[:N_NODES]          # [10000, 288]
    out = full.reshape(N_NODES, NSH, C).transpose(0, 2, 1)  # [10000, 32, 9]
    return np.ascontiguousarray(out.astype(np.float32))


# revision 51
# speedup vs baseline: 1.2130x; 1.0736x over previous
"""MessagePassingConvolution kernel for 8 Trainium2 NeuronCores.

Strategy (no collectives; ~347us profiled on HW):
  - Host: sort edges by receiver; shard by receiver windows. Core m owns
    nodes [m*1280, (m+1)*1280) = 10 windows of 128 nodes. Each window's
    edge list is padded to a fixed budget (2176 = 17 subtiles of 128) so
    the SPMD program is identical across cores. The sender gather
    (node_feats[senders]) is done host-side.
  - Einsum u[e,lo] = sum_{k,i} h[e,k] x[e,i] Wgen[k,lo,i] runs as 16
    chunk matmuls over a materialized outer product A[e,(k,i)].  The
    (k,i)->partition/chunk mapping is chosen to minimize operand
    replication DMA: k = 4*ks + ck, i = 8*ci + is, partition p = 8ks+is,
    chunk (ck,ci).  h3 is replicated 8x (DRAM-bounce broadcast, 1KB/edge)
    and x 16x (free: baked into the host-gathered stream); the DVE builds
    A in one bf16 2x-mode op per 512 edges with both factors read via
    free-dim broadcasts.  The broadcast DMA's source APs put a >=16-size
    dim outermost, which is what makes the DGE spread descriptors over
    all 16 SDMA engines (outer dim 4 -> only 4 engines, 4x slower).
  - All matmuls bf16 (fp32 matmul runs LOW/HIGH double passes, ~3x cost).
  - Per (window, 1024-edge pair), software-pipelined with a 1-pair skew
    (stage A: loads + MLP + bounce; stage B: outer product + einsum +
    transposes) so load chains complete while the previous pair computes:
      MLP (3 bf16 matmuls + Silu) -> h3p [64, e] bf16
      h3mid[p, ck, e] = h3p[4ks+ck, e] via one broadcast DMA per pair
      A = h3mid*x (DVE), u[96,*] += Wg_c.T @ A_c (16 matmuls/512)
      tensor-engine transpose -> ut (bf16), msgs = ut * attrs (DVE)
      scatter: psum[128n, 288] += S.T @ msgs, S built on-device from
        recv_local via iota==scalar compare (one DVE op per window)
    The window's tail-subtile msgs + scatter + store are deferred into
    the next window so the DVE never stalls on the tail chain.
  - Output: per-core [1280, 288] slices -> concat -> [10000, 32, 9].
"""

import sys
import numpy as np
from contextlib import ExitStack

sys.path.insert(0, "/opt/trn_rl_repo")

import concourse.bass as bass  # noqa: E402
import concourse.bacc as bacc  # noqa: E402
import concourse.mybir as mybir  # noqa: E402
import concourse.tile as tile  # noqa: E402
from concourse.masks import make_identity  # noqa: E402
from concourse.bass_utils import run_bass_kernel_spmd  # noqa: E402

import ml_dtypes  # noqa: E402

BF16 = ml_dtypes.bfloat16

# ---- problem constants (hardcoded per spec) ----
N_NODES = 10000
N_EDGES = 160000
C = 32
RADIAL = 8
HID = 64
NL = 3
L_DIMS = (1, 3, 5)
NSH = 9  # sum(L_DIMS)
AVG_NUM_NEIGHBORS = 16.0

N_CORES = 8
WIN = 128                      # nodes per window (psum partitions)
WINS_PER_CORE = 10
NODES_PER_CORE = WIN * WINS_PER_CORE     # 1280
N_NODES_PAD = NODES_PER_CORE * N_CORES   # 10240
SUB = 128                      # edges per subtile
SUBS_PER_WIN = 17              # window edge budget = 2176 (data max 2155)
WIN_E = SUB * SUBS_PER_WIN     # 2176
E_CORE = WIN_E * WINS_PER_CORE  # 21760
N_ST = E_CORE // SUB           # 170 subtiles per core
TILE_SIZES = (512, 512, 512, 512, 128)   # per-window einsum tiles
N_CHUNK = 16                   # ki chunks (2048 / 128)
LO = NL * C                    # 96
F_OUT = NSH * C                # 288

FP32 = mybir.dt.float32
BF16_DT = mybir.dt.bfloat16

_CACHED = {}

# CoreSim doesn't implement Silu; sim_test.py overrides this to Sigmoid and
# checks against a sigmoid-variant reference to validate the data plumbing.
ACT_FUNC = mybir.ActivationFunctionType.Silu


def _build_nc():
    nc = bacc.Bacc()

    ef = nc.dram_tensor("ef", [RADIAL, E_CORE], BF16_DT, kind="ExternalInput")
    x4 = nc.dram_tensor("x4", [128, 4 * E_CORE], BF16_DT, kind="ExternalInput")
    at = nc.dram_tensor("at", [SUB, N_ST * F_OUT], BF16_DT,
                        kind="ExternalInput")
    rl = nc.dram_tensor("rl", [SUB, N_ST], FP32, kind="ExternalInput")
    w1 = nc.dram_tensor("w1", [RADIAL, HID], BF16_DT, kind="ExternalInput")
    w2 = nc.dram_tensor("w2", [HID, HID], BF16_DT, kind="ExternalInput")
    w3 = nc.dram_tensor("w3", [HID, HID], BF16_DT, kind="ExternalInput")
    wg = nc.dram_tensor("wg", [128, N_CHUNK * LO], BF16_DT, kind="ExternalInput")
    iota = nc.dram_tensor("iota", [128, 128], FP32, kind="ExternalInput")
    out = nc.dram_tensor("out", [NODES_PER_CORE, F_OUT], FP32, kind="ExternalOutput")

    with tile.TileContext(nc) as tc, ExitStack() as ctx:
        const_p = ctx.enter_context(tc.tile_pool(name="const", bufs=1))
        stream_p = ctx.enter_context(tc.tile_pool(name="stream", bufs=3))
        win_p = ctx.enter_context(tc.tile_pool(name="win", bufs=2))
        chunk_p = ctx.enter_context(tc.tile_pool(name="chunk", bufs=3))
        bc_p = ctx.enter_context(tc.tile_pool(name="bc", bufs=3))
        psum_mlp = ctx.enter_context(tc.tile_pool(name="pmlp", bufs=2, space="PSUM"))
        psum_u = ctx.enter_context(tc.tile_pool(name="pu", bufs=3, space="PSUM"))
        psum_ut = ctx.enter_context(tc.tile_pool(name="put", bufs=2, space="PSUM"))
        psum_acc = ctx.enter_context(tc.tile_pool(name="pacc", bufs=1, space="PSUM"))
        dram_p = ctx.enter_context(tc.tile_pool(name="dram", bufs=3, space="DRAM"))
        tail_p = ctx.enter_context(tc.tile_pool(name="tail", bufs=1))

        # ---- one-time constants into SBUF ----
        w1_sb = const_p.tile([RADIAL, HID], BF16_DT)
        nc.scalar.dma_start(w1_sb[:], w1[:])
        w2_sb = const_p.tile([2 * HID, HID], BF16_DT)
        nc.scalar.dma_start(w2_sb[:HID], w2[:])
        nc.scalar.dma_start(w2_sb[HID:], w2[:])
        w3_sb = const_p.tile([2 * HID, HID], BF16_DT)
        nc.scalar.dma_start(w3_sb[:HID], w3[:])
        nc.scalar.dma_start(w3_sb[HID:], w3[:])
        wg_sb = const_p.tile([128, N_CHUNK * LO], BF16_DT)
        nc.scalar.dma_start(wg_sb[:], wg[:])
        iota_sb = const_p.tile([128, 128], FP32)
        nc.scalar.dma_start(iota_sb[:], iota[:])
        ident_sb = const_p.tile([128, 128], BF16_DT)
        make_identity(nc, ident_sb[:])

        lofs = (0, 1, 4)

        def msgs_range(f_ut, f_at, f_msgs, s0, s1):
            # msgs = ut * attr-expanded for subtiles [s0, s1); attrs are
            # host-expanded along c so both DVE reads are packed step-1
            # bf16 and the op runs in 2x mode
            ns = s1 - s0
            for l in range(NL):
                dim = L_DIMS[l]
                u_ap = f_ut[:, s0:s1, None, l * C:(l + 1) * C].to_broadcast(
                    [SUB, ns, dim, C])
                a_ap = f_at[:].rearrange(
                    "p (s m c) -> p s m c", m=NSH, c=C)[
                        :, s0:s1, lofs[l]:lofs[l] + dim, :]
                nc.vector.tensor_tensor(
                    out=f_msgs[:, s0:s1, lofs[l] * C:(lofs[l] + dim) * C]
                    .rearrange("p s (m c) -> p s m c", c=C),
                    in0=u_ap, in1=a_ap, op=mybir.AluOpType.mult)

        def flush_window(fw, f_ut, f_at, f_msgs, f_s):
            # tail subtile's msgs, then scatter + store
            msgs_range(f_ut, f_at, f_msgs, SUBS_PER_WIN - 1, SUBS_PER_WIN)
            # scatter: psum_out += S.T @ msgs per subtile
            acc = psum_acc.tile([WIN, F_OUT], FP32, tag="acc")
            for st in range(SUBS_PER_WIN):
                nc.tensor.matmul(out=acc[:], lhsT=f_s[:, st, :],
                                 rhs=f_msgs[:, st, :],
                                 start=(st == 0), stop=(st == SUBS_PER_WIN - 1),
                                 skip_group_check=True)
            out_sb = stream_p.tile([WIN, F_OUT], FP32, tag="osb")
            nc.scalar.copy(out_sb[:], acc[:])
            nc.scalar.dma_start(out[fw * WIN:(fw + 1) * WIN, :], out_sb[:])

        # ---- flat skewed pipeline over all (window, pair) tiles ----
        # stage A issues DMAs + MLP + bounce for a pair; stage B (one step
        # later) does the outer product + einsum + transposes, so each
        # pair's load chain completes while the previous pair computes.
        win_tiles = {}

        def win_start(w):
            at_sb = win_p.tile([SUB, SUBS_PER_WIN * F_OUT], BF16_DT, tag="at")
            nc.scalar.dma_start(
                at_sb[:],
                at[:, w * SUBS_PER_WIN * F_OUT:(w + 1) * SUBS_PER_WIN * F_OUT])
            rl_sb = win_p.tile([SUB, SUBS_PER_WIN], FP32, tag="rl")
            nc.scalar.dma_start(
                rl_sb[:], rl[:, w * SUBS_PER_WIN:(w + 1) * SUBS_PER_WIN])
            ut_sb = win_p.tile([SUB, SUBS_PER_WIN, LO], BF16_DT, tag="ut")
            msgs_sb = win_p.tile([SUB, SUBS_PER_WIN, F_OUT], BF16_DT, tag="msgs")
            s_all = win_p.tile([SUB, SUBS_PER_WIN, WIN], BF16_DT, tag="s")
            nc.vector.tensor_tensor(
                out=s_all[:],
                in0=iota_sb[:, None, :].to_broadcast([SUB, SUBS_PER_WIN, WIN]),
                in1=rl_sb[:, :, None].to_broadcast([SUB, SUBS_PER_WIN, WIN]),
                op=mybir.AluOpType.is_equal)
            win_tiles[w] = (at_sb, ut_sb, msgs_sb, s_all)

        def stage_a(w, e_off, psz):
            base = w * WIN_E + e_off
            halves = (512, 512) if psz == 1024 else (psz,)
            ef_sb = stream_p.tile([RADIAL, psz], BF16_DT, tag=f"ef{psz}")
            nc.sync.dma_start(ef_sb[:], ef[:, base:base + psz])
            # Xrep[p=(ks,is), ci, e] = x_s[e, 8*ci + is] (host-gathered,
            # pair-blocked so each partition is one contiguous run; on the
            # sync queue so load issue never queues behind activations)
            xoff = 4 * base
            x_sb = stream_p.tile([128, 4, psz], BF16_DT, tag=f"x{psz}")
            nc.sync.dma_start(x_sb[:].rearrange("p c e -> p (c e)"),
                              x4[:, xoff:xoff + 4 * psz])

            # MLP with the pair's two 512-halves stacked on partition
            # ranges [0:64] / [64:128]: one activation per layer per pair
            h3d = dram_p.tile([HID, psz], BF16_DT, tag=f"h3d{psz}")
            nh = len(halves)
            hw = halves[0]
            z1 = psum_mlp.tile([128, 512], FP32, tag="z")
            for i in range(nh):
                nc.tensor.matmul(out=z1[64 * i:64 * i + HID, :hw],
                                 lhsT=w1_sb[:],
                                 rhs=ef_sb[:, i * hw:(i + 1) * hw],
                                 start=True, stop=True, skip_group_check=True)
            h1 = stream_p.tile([128, 512], BF16_DT, tag="h1")
            nc.scalar.activation(h1[:64 * nh, :hw], z1[:64 * nh, :hw], ACT_FUNC)
            z2 = psum_mlp.tile([128, 512], FP32, tag="z")
            for i in range(nh):
                nc.tensor.matmul(out=z2[64 * i:64 * i + HID, :hw],
                                 lhsT=w2_sb[64 * i:64 * i + HID],
                                 rhs=h1[64 * i:64 * i + HID, :hw],
                                 start=True, stop=True, skip_group_check=True)
            h2 = stream_p.tile([128, 512], BF16_DT, tag="h2")
            nc.scalar.activation(h2[:64 * nh, :hw], z2[:64 * nh, :hw], ACT_FUNC)
            z3 = psum_mlp.tile([128, 512], FP32, tag="z")
            for i in range(nh):
                nc.tensor.matmul(out=z3[64 * i:64 * i + HID, :hw],
                                 lhsT=w3_sb[64 * i:64 * i + HID],
                                 rhs=h2[64 * i:64 * i + HID, :hw],
                                 start=True, stop=True, skip_group_check=True)
            h3p = stream_p.tile([128, 512], BF16_DT, tag="h3")
            nc.scalar.activation(h3p[:64 * nh, :hw], z3[:64 * nh, :hw], ACT_FUNC)
            for i in range(nh):
                nc.sync.dma_start(h3d[:, i * hw:(i + 1) * hw],
                                  h3p[64 * i:64 * i + HID, :hw])

            # broadcast: h3mid[p=8ks+is, ck, e] = h3p[4ks + ck, e] (8x rep;
            # src outer dim 16 spreads descriptors over 16 SDMA engines)
            pool = bc_p if psz == 1024 else tail_p
            h3mid = pool.tile([128, 4, psz], BF16_DT, tag=f"h3bc{psz}")
            dst = h3mid[:].rearrange("p c e -> p (c e)")
            src2 = h3d[:].rearrange("(ks ck) e -> ks (ck e)", ks=16)
            src2 = src2[:, None, :].to_broadcast([16, 8, 4 * psz])
            nc.sync.dma_start(dst, src2)
            return (w, e_off, psz, x_sb, h3mid)

        def stage_b(stt):
            w, e_off, psz, x_sb, h3mid = stt
            at_sb, ut_sb, msgs_sb, s_all = win_tiles[w]
            halves = (512, 512) if psz == 1024 else (psz,)
            h0 = 0
            for hsz in halves:
                hs = slice(h0, h0 + hsz)
                a_all = (chunk_p if hsz == 512 else tail_p).tile(
                    [128, N_CHUNK, hsz], BF16_DT, tag=f"a{hsz}")
                nc.vector.tensor_tensor(
                    out=a_all[:].rearrange("p (ck ci) e -> p ck ci e", ck=4),
                    in0=h3mid[:, :, None, hs].to_broadcast([128, 4, 4, hsz]),
                    in1=x_sb[:, None, :, hs].to_broadcast([128, 4, 4, hsz]),
                    op=mybir.AluOpType.mult)
                u_ps = psum_u.tile([LO, 512], FP32, tag="u")
                for c in range(N_CHUNK):
                    nc.tensor.matmul(out=u_ps[:, :hsz],
                                     lhsT=wg_sb[:, c * LO:(c + 1) * LO],
                                     rhs=a_all[:, c, :],
                                     start=(c == 0), stop=(c == N_CHUNK - 1),
                                     skip_group_check=True)
                u_sb = stream_p.tile([LO, 512], BF16_DT, tag="usb")
                nc.scalar.copy(u_sb[:, :hsz], u_ps[:, :hsz])
                ut_ps = psum_ut.tile([128, 4, LO], BF16_DT, tag="utp")
                nsub = hsz // SUB
                for s in range(nsub):
                    nc.tensor.transpose(
                        out=ut_ps[:, s, :],
                        in_=u_sb[:, s * SUB:(s + 1) * SUB],
                        identity=ident_sb[:LO, :LO])
                st0 = (e_off + h0) // SUB
                nc.scalar.copy(ut_sb[:, st0:st0 + nsub, :], ut_ps[:, :nsub, :])
                h0 += hsz
            if psz == 1024:
                msgs_range(ut_sb, at_sb, msgs_sb, e_off // SUB,
                           (e_off + psz) // SUB)

        tiles = [(w, e_off, psz) for w in range(WINS_PER_CORE)
                 for (e_off, psz) in ((0, 1024), (1024, 1024), (2048, 128))]
        SKEW = 2
        inflight = []
        for (w, e_off, psz) in tiles:
            if e_off == 0:
                win_start(w)
            inflight.append(stage_a(w, e_off, psz))
            if len(inflight) > SKEW:
                prev = inflight.pop(0)
                stage_b(prev)
                if prev[2] == 128:  # last pair of its window
                    pw = prev[0]
                    wt = win_tiles.pop(pw)
                    flush_window(pw, wt[1], wt[0], wt[2], wt[3])
        for prev in inflight:
            stage_b(prev)
            if prev[2] == 128:
                pw = prev[0]
                wt = win_tiles.pop(pw)
                flush_window(pw, wt[1], wt[0], wt[2], wt[3])

    nc.compile()
    return nc


def _host_prep(node_feats, edge_attrs, edge_feats, senders, receivers,
               W1, W2, W3, Wgen):
    """Sort/shard edges by receiver window, build per-core input maps."""
    senders = np.asarray(senders).astype(np.int64)
    receivers = np.asarray(receivers).astype(np.int64)
    node_feats = np.asarray(node_feats, dtype=np.float32)
    edge_attrs = np.asarray(edge_attrs, dtype=np.float32)
    edge_feats = np.asarray(edge_feats, dtype=np.float32)

    n_win_total = N_CORES * WINS_PER_CORE  # 80
    win_id = receivers // WIN
    order = np.argsort(win_id, kind="stable")
    counts = np.bincount(win_id, minlength=n_win_total)
    assert counts.max() <= WIN_E, f"window overflow: {counts.max()} > {WIN_E}"
    starts = np.zeros(n_win_total + 1, np.int64)
    np.cumsum(counts, out=starts[1:])

    # slot arrays (padded); padding edges: ef=0, attr=0 -> msgs contribution 0
    E_TOT = N_CORES * E_CORE
    ef_s = np.zeros((E_TOT, RADIAL), np.float32)
    at_s = np.zeros((E_TOT, NSH), np.float32)
    rl_s = np.zeros(E_TOT, np.float32)
    sd_s = np.zeros(E_TOT, np.int64)

    slot_base = np.arange(n_win_total) * WIN_E
    # positions for real edges
    within = np.arange(len(order)) - starts[win_id[order]]
    slots = slot_base[win_id[order]] + within
    ef_s[slots] = edge_feats[order]
    at_s[slots] = edge_attrs[order] * np.float32(1.0 / np.sqrt(AVG_NUM_NEIGHBORS))
    rl_s[slots] = (receivers[order] % WIN).astype(np.float32)
    sd_s[slots] = senders[order]

    # host-side sender gather, replicated 4x along partitions (bf16)
    xg = node_feats[sd_s].astype(BF16)            # [E_TOT, 32]

    # weights with fan-in scales folded
    w1 = (W1 * (1.0 / np.sqrt(RADIAL))).astype(BF16)
    w2 = (W2 * (1.0 / np.sqrt(HID))).astype(BF16)
    w3 = (W3 * (1.0 / np.sqrt(HID))).astype(np.float32)  # permuted below, cast after
    w3p = np.ascontiguousarray(w3).astype(BF16)
    # p = 8*ks + is, chunk c = 4*ck + ci: wg[c][p, lo] =
    #   Wgen[4*ks + ck, l, o, 8*ci + is] / sqrt(HID*C)
    wgen = np.asarray(Wgen, dtype=np.float32) * np.float32(1.0 / np.sqrt(HID * C))
    p = np.arange(128)
    ks, is_ = p // 8, p % 8
    wgc = np.zeros((N_CHUNK, 128, NL, C), np.float32)
    for ck in range(4):
        for ci in range(4):
            wgc[4 * ck + ci] = wgen[4 * ks + ck][p, :, :, 8 * ci + is_].reshape(
                128, NL, C)
    # -> [128, 16*96]: chunk-major along free dim
    wgc = wgc.reshape(N_CHUNK, 128, LO).transpose(1, 0, 2).reshape(
        128, N_CHUNK * LO)
    wgc = wgc.astype(BF16)

    iota = np.broadcast_to(np.arange(128, dtype=np.float32),
                           (128, 128)).copy()

    in_maps = []
    for m in range(N_CORES):
        sl = slice(m * E_CORE, (m + 1) * E_CORE)
        ef_c = ef_s[sl]      # [E_CORE, 8]
        at_c = at_s[sl]      # [E_CORE, 9]
        rl_c = rl_s[sl]
        # attrs expanded along c (col m*C + j = attr[m]), bf16, subtile-major
        atc = np.ascontiguousarray(
            np.repeat(at_c, C, axis=1).astype(BF16).reshape(
                N_ST, SUB, F_OUT).transpose(1, 0, 2).reshape(
                    SUB, N_ST * F_OUT))
        # x4m[p=(ks,is), ci, e] = xg[e, 8*ci + is], ks-replicated 16x,
        # then pair-blocked: per (window, pair) a contiguous [4*psz] run
        t = xg[sl].T.reshape(4, 8, E_CORE).transpose(1, 0, 2)  # [is, ci, e]
        x4f = np.broadcast_to(t[None], (16, 8, 4, E_CORE)).reshape(
            128, 4, E_CORE)
        blocks = []
        for wi in range(WINS_PER_CORE):
            for eo, psz in ((0, 1024), (1024, 1024), (2048, 128)):
                b = wi * WIN_E + eo
                blocks.append(x4f[:, :, b:b + psz].reshape(128, 4 * psz))
        x4_c = np.ascontiguousarray(np.concatenate(blocks, axis=1))
        in_maps.append({
            "ef": np.ascontiguousarray(ef_c.T.astype(BF16)),
            "at": atc,
            "rl": np.ascontiguousarray(
                rl_c.reshape(N_ST, SUB).T),
            "x4": x4_c,
            "w1": w1, "w2": w2, "w3": w3p, "wg": wgc,
            "iota": iota,
        })
    return in_maps


def kernel(node_feats, edge_attrs, edge_feats, senders, receivers,
           W1, W2, W3, Wgen):
    in_maps = _host_prep(node_feats, edge_attrs, edge_feats, senders, receivers,
                         W1, W2, W3, Wgen)
    if "nc" not in _CACHED:
        _CACHED["nc"] = _build_nc()
    nc = _CACHED["nc"]
    res = run_bass_kernel_spmd(nc, in_maps, core_ids=list(range(N_CORES)))
    outs = [res.results[m]["out"] for m in range(N_CORES)]
    full = np.concatenate(outs, axis=0)[:N_NODES]          # [10000, 288]
    out = full.reshape(N_NODES, NSH, C).transpose(0, 2, 1)  # [10000, 32, 9]
    return np.ascontiguousarray(out.astype(np.float32))
